# revision 30
# baseline (speedup 1.0000x reference)
"""Locally-connected graph-conv kernel for Trainium2 (Bass/Tile).

Computes out[b,t,m] = sum_n x[b,t,n] * (S*W)[n,m] + bias[m] for
x [64, 2048, 208], W/S [208, 208], bias [208].

The ring-graph support S is a +-4 band (mod 208): output node m only
depends on x nodes m-4..m+4. The 208 outputs are split into FOUR
groups of 52, each needing a 60-row contraction slice, and the four
[60,52] premasked weight tiles are packed into the 2x2 quadrants of
the 128x128 PE array via tile_position:
    G0 outs   0.. 51  rot rows   0.. 59  quadrant (0,0)    x-tile E
    G1 outs  52..103  rot rows  52..111  quadrant (64,64)  x-tile E
    G2 outs 104..155  rot rows 104..163  quadrant (0,64)   x-tile O
    G3 outs 156..207  rot rows 156..215  quadrant (64,0)   x-tile O
(rot row j = node (j-4) mod 208). Per 512 t-columns, FOUR matmuls run
CONCURRENTLY in the four quadrants (hardware per-subarray concurrency;
LDWEIGHTS for one quadrant overlaps in-flight matmuls in others), so
every t-column is streamed once per x-tile instead of once per
104-output block: ~2x the PE throughput of the 2-block layout and fast
enough (~2.2 us per 2048 cols even at the cold HAM clock) that the PE
never paces the DMA pipeline - no HAM warm-up games needed.

Quadrant alignment wants G0/G2 rows at SBUF partitions 0:60 and G1/G3
at 64:124 (matmul moving-operand base partition must equal the
tile_position row), and results land at PSUM partitions 0:52 and
64:116. All HBM transfers use plain full-128-partition shapes - the
host pads the x streams to 128 rows and unpacks the out streams from
rows {0:52, 64:116} - because (measured) two-level partition APs
silently corrupt DMA addressing and ragged partition counts fall off
the DMA fast path (116-row SWDGE stores: 141 GB/s vs 220+ at 128).

Everything that touches HBM is bf16 (PSUM accumulation stays fp32).
Measured HW behavior this build is tuned against:
 - ONE HWDGE ring sustains only ~260 GB/s, and a DMA issue that waits
   on a compute semaphore BLOCKS the issuing engine. So ALL load
   issues go up-front (Sync: wh then xE chunks; Scalar: bias then xO
   chunks; engines never block while their ring still has load bytes)
   and the rings drain pure-read at a measured 370-420 GB/s combined
   until every chunk lands (~38 us). ALL stores ride the GpSimd SWDGE
   ring (~250 GB/s at clean shapes), which fully owns the write
   stream; routing tail stores onto the HWDGE rings measured slower.
   Ragged partition counts must be avoided (116-row stores: 99-141
   GB/s) - every transfer here is a full 128 partitions.
 - The PE can sit at half clock for 10-20 us stretches mid-kernel
   (package power throttling, not just HAM warm-up - dense matmul
   streams measured at K=4/8 for 16 us straight), so the design
   assumes the COLD clock: quadrant-packed matmuls need ~2.6 us per
   2048 cols cold, comfortably under the ~3.3 us DMA chunk cadence.
   No warm-up or keep-alive dummies needed.
 - chunk sizes taper at BOTH ends: small first chunks start compute
   early, small last chunks keep the store tail short.
 - PSUM->SBUF eviction is 1 elem/lane/cycle (fp32 source): the E
   bank-pair evicts on VectorE, the O bank-pair on ScalarE, one
   [116,x] op per bank-pair, fusing bias and the fp32->bf16 convert.
The host transposes y^T back at gather.
"""

import numpy as np
import ml_dtypes
from contextlib import ExitStack

import concourse.bacc as bacc
import concourse.mybir as mybir
import concourse.tile as tile
from concourse.bass_utils import run_bass_kernel_spmd

N = 208                      # nodes
K = 4                        # band half-width of S
G = 52                       # output nodes per PE quadrant tile
GR = G + 2 * K               # 60 contraction rows per group
QP = 64                      # quadrant partition pitch
GE = QP + GR                 # 124 used partitions per x tile
EV = QP + G                  # 116 evicted partitions per bank-pair
XROWS = 128                  # x/out tile + DRAM stream partition count
WPAD = 1024                  # wh DRAM row padding (2 KB rows -> fast DMA)
BPAD = 256                   # bias DRAM row padding (1 KB f32 rows)
N_CORES = 8
B, T = 64, 2048
ROWS_TOTAL = B * T           # 131072
SHARD = ROWS_TOTAL // N_CORES    # 16384 rows per core
TB = 512                     # moving-block columns per matmul (fp32 PSUM max)
TB2 = 2 * TB                 # eviction group (2 PSUM banks)
CHUNKS = [1024, 1024, 2048, 2048, 2048, 2048, 2048, 1024, 1024, 1024, 1024]
assert sum(CHUNKS) == SHARD
STORE_SPLIT = 8              # chunks storing via SWDGE; rest via HWDGE
                             # (split=6 at bufs=3 measured slower -
                             # those stores collided with the load
                             # tail; with bufs=6 only the last taper
                             # chunks move, draining on the
                             # by-then-idle HWDGE rings while the
                             # SWDGE queue shortens by the same bytes)

FP32 = mybir.dt.float32
BF16 = mybir.dt.bfloat16
NP_BF16 = ml_dtypes.bfloat16
IDENT = mybir.ActivationFunctionType.Identity

_CACHE = {}
LAST_RESULTS = None          # BassKernelResults of the most recent run


def _kernel_body(tc):
    nc = tc.nc
    x_e = nc.dram_tensor("xe", [XROWS, SHARD], BF16, kind="ExternalInput").ap()
    x_o = nc.dram_tensor("xo", [XROWS, SHARD], BF16, kind="ExternalInput").ap()
    w_d = nc.dram_tensor("wh", [XROWS, WPAD], BF16, kind="ExternalInput").ap()
    b_d = nc.dram_tensor("bias", [XROWS, BPAD], FP32, kind="ExternalInput").ap()
    o_d = nc.dram_tensor("outt", [2 * XROWS, SHARD], BF16, kind="ExternalOutput").ap()

    with ExitStack() as ctx:
        const = ctx.enter_context(tc.tile_pool(name="const", bufs=1))

        # Ring heads: wh leads Sync, bias leads Scalar (both tiny, done
        # in <1 us at the head of their FIFOs).
        wh = const.tile([XROWS, WPAD], BF16, tag="wh")
        nc.sync.dma_start(wh, w_d)
        bt = const.tile([XROWS, BPAD], FP32, tag="bt")
        nc.scalar.dma_start(bt, b_d)
        bAc = bt[0:EV, 0:1]
        bBc = bt[0:EV, 1:2]

        # bufs=6: the SWDGE store drain (~250 GB/s) lags the eviction
        # rate; at bufs=3 the pool recycle (evict c+3 waits store c)
        # backpressured evictions and matmuls to the store pace
        # (measured: matmuls ran 14 us past the last load). Six
        # buffers (~24 KB/partition for both pools) let the compute
        # side finish right behind the loads while stores drain.
        oAp = ctx.enter_context(tc.tile_pool(name="oAp", bufs=6))
        oBp = ctx.enter_context(tc.tile_pool(name="oBp", bufs=6))
        psAp = ctx.enter_context(tc.tile_pool(name="psAp", bufs=2, space="PSUM"))
        psBp = ctx.enter_context(tc.tile_pool(name="psBp", bufs=2, space="PSUM"))

        # Load issues run PREF chunks ahead of the store issues below,
        # so each HWDGE ring FIFO alternates [store c][load c+PREF]
        # bytes: reads and writes interleave on both rings (the regime
        # measured at ~400+ GB/s combined) and a store's eviction-sem
        # wait never leaves its ring without queued load bytes.
        xts = []
        col = 0
        for c, csz in enumerate(CHUNKS):
            xe = const.tile([XROWS, csz], BF16, tag=f"xe_{c}")
            xo = const.tile([XROWS, csz], BF16, tag=f"xo_{c}")
            xts.append((xe, xo, col, csz))
            col += csz

        # ALL load issues up-front: engines never block on compute
        # sems while their ring has load bytes to move, so both HWDGE
        # rings drain continuously until every x chunk has landed.
        for xe, xo, col, csz in xts:
            nc.sync.dma_start(xe, x_e[:, col : col + csz])
        for xe, xo, col, csz in xts:
            nc.scalar.dma_start(xo, x_o[:, col : col + csz])

        for c, (xe, xo, col, csz) in enumerate(xts):
            tsl = slice(col, col + csz)
            oA_t = oAp.tile([XROWS, csz], BF16, tag="oA")
            oB_t = oBp.tile([XROWS, csz], BF16, tag="oB")
            for s in range((csz + TB2 - 1) // TB2):
                g0 = s * TB2
                gw = min(TB2, csz - g0)
                g = slice(g0, g0 + gw)
                # [128, 1024] PSUM tiles (2 banks); each of the four
                # quadrant matmuls targets one bank, partitions 0:52 /
                # 64:116.
                psA = psAp.tile([XROWS, TB2], FP32, tag="psA")
                psB = psBp.tile([XROWS, TB2], FP32, tag="psB")
                for q0 in range(0, gw, TB):
                    qs = slice(g0 + q0, g0 + q0 + TB)
                    qp = slice(q0, q0 + TB)
                    nc.tensor.matmul(psA[0:G, qp], wh[0:GR, 0:G],
                                     xe[0:GR, qs], start=True, stop=True,
                                     tile_position=(0, 0))
                    nc.tensor.matmul(psA[QP:EV, qp], wh[QP:GE, 0:G],
                                     xe[QP:GE, qs], start=True, stop=True,
                                     tile_position=(QP, QP))
                    nc.tensor.matmul(psB[QP:EV, qp], wh[0:GR, G : 2 * G],
                                     xo[0:GR, qs], start=True, stop=True,
                                     tile_position=(0, QP))
                    nc.tensor.matmul(psB[0:G, qp], wh[QP:GE, G : 2 * G],
                                     xo[QP:GE, qs], start=True, stop=True,
                                     tile_position=(QP, 0))
                # one [116,gw] eviction per bank-pair moves both groups
                # (junk partitions 52:64 ride along); bias + fp32->bf16
                # fused. E pair on VectorE, O pair on ScalarE.
                nc.vector.tensor_scalar_add(oA_t[0:EV, g], psA[0:EV, 0:gw], bAc)
                nc.scalar.activation(oB_t[0:EV, g], psB[0:EV, 0:gw], IDENT, bias=bBc)
            # Early-chunk stores ride the GpSimd SWDGE ring (own FIFO,
            # clean 128-partition shapes run at ~250 GB/s there), so
            # blocking on eviction sems never starves a load ring and
            # the HWDGE rings stay pure-read while loads flow. The
            # taper chunks' stores go to Sync/Scalar instead - their
            # descriptors queue AFTER all load bytes, so they drain on
            # the otherwise-idle HWDGE rings right as loads finish
            # (measured: one SWDGE ring alone drags the store tail to
            # ~57 us while both HWDGE rings idle from ~38 us).
            if c < STORE_SPLIT:
                nc.gpsimd.dma_start(o_d[0:XROWS, tsl], oA_t)
                nc.gpsimd.dma_start(o_d[XROWS : 2 * XROWS, tsl], oB_t)
            else:
                nc.sync.dma_start(o_d[0:XROWS, tsl], oA_t)
                nc.scalar.dma_start(o_d[XROWS : 2 * XROWS, tsl], oB_t)


def _build():
    nc = bacc.Bacc(
        "TRN2",
        target_bir_lowering=False,
        debug=False,
        num_devices=N_CORES,
    )
    with tile.TileContext(nc) as tc:
        _kernel_body(tc)
    nc.compile()
    return nc


def kernel(x, W, b, S):
    global LAST_RESULTS
    nc = _CACHE.get("nc")
    if nc is None:
        nc = _build()
        _CACHE["nc"] = nc

    xf = np.asarray(x, np.float32).reshape(ROWS_TOTAL, N)
    SW = (np.asarray(S, np.float32) * np.asarray(W, np.float32))
    rot = [(r - K) % N for r in range(N + 2 * K)]       # rot row -> node
    SWr = SW[rot, :]                                    # [216, 208]
    wh = np.zeros((XROWS, WPAD), NP_BF16)
    wh[0:GR, 0:G] = SWr[0:GR, 0:G]                      # G0
    wh[QP:GE, 0:G] = SWr[G : G + GR, G : 2 * G]         # G1
    wh[0:GR, G : 2 * G] = SWr[2 * G : 2 * G + GR, 2 * G : 3 * G]   # G2
    wh[QP:GE, G : 2 * G] = SWr[3 * G : 3 * G + GR, 3 * G : 4 * G]  # G3
    bfv = np.asarray(b, np.float32).reshape(N)
    bf = np.zeros((XROWS, BPAD), np.float32)
    bf[0:G, 0] = bfv[0:G]                # E pair col 0: G0 at 0:52
    bf[QP:EV, 0] = bfv[G : 2 * G]        #               G1 at 64:116
    bf[0:G, 1] = bfv[3 * G : 4 * G]      # O pair col 1: G3 at 0:52
    bf[QP:EV, 1] = bfv[2 * G : 3 * G]    #               G2 at 64:116

    in_maps = []
    for i in range(N_CORES):
        xt = xf[i * SHARD : (i + 1) * SHARD].T          # [208, SHARD] view
        xr = np.empty((N + 2 * K, SHARD), NP_BF16)      # rotated rows
        xr[0:K] = xt[N - K : N]
        xr[K : N + K] = xt
        xr[N + K :] = xt[0:K]
        xe = np.zeros((XROWS, SHARD), NP_BF16)
        xe[0:GR] = xr[0:GR]                             # G0 rows
        xe[QP:GE] = xr[G : G + GR]                      # G1 rows
        xo = np.zeros((XROWS, SHARD), NP_BF16)
        xo[0:GR] = xr[2 * G : 2 * G + GR]               # G2 rows
        xo[QP:GE] = xr[3 * G : 3 * G + GR]              # G3 rows
        in_maps.append({"xe": xe, "xo": xo, "wh": wh, "bias": bf})
    res = run_bass_kernel_spmd(nc, in_maps, core_ids=list(range(N_CORES)))
    LAST_RESULTS = res
    out = np.empty((ROWS_TOTAL, N), np.float32)
    for i, r in enumerate(res.results):
        yt = r["outt"]                                  # [256, SHARD] bf16
        sl = slice(i * SHARD, (i + 1) * SHARD)
        out[sl, 0:G] = yt[0:G].T                        # G0
        out[sl, G : 2 * G] = yt[QP:EV].T                # G1
        out[sl, 3 * G : 4 * G] = yt[XROWS : XROWS + G].T        # G3
        out[sl, 2 * G : 3 * G] = yt[XROWS + QP : XROWS + EV].T  # G2
    return out.reshape(B, T, N)
